# revision 6
# baseline (speedup 1.0000x reference)
"""GQA causal attention (B=2,S=2048,HID=2048,H=16,KVH=4,D=128) on 8 trn2 cores.

Sharding: core = b*4 + g  (b: batch, g: head-group of 4 Q heads + 1 KV head).
Per-core kernel computes q/k/v projections (+RoPE), causal softmax attention
for its 4 heads, and a partial output projection; host sums the 4 partials
per batch.

v2 (pipelined, fp16):
  - All matmul operands fp16 (same PE speed as bf16, ~8x mantissa) with fp32
    PSUM accumulation; output partials fp16, host sums in f32.
  - Startup DMAs split into 4-k-group pieces so the first projection matmuls
    start ~2us in instead of waiting for whole-tensor loads.
  - Attention emitted head-interleaved per key-block (4 independent
    QK->exp->AV chains; avT holds 4 PSUM banks) so PE never stalls on the
    ACT exp of a single chain.
  - proj(c+1) emission spread between attention(c) key-block rounds: PE has
    dense independent work while ACT drains the exp queue.
  - softmax denominator accumulation: heads 0/1 on DVE in fp16 (2x mode),
    heads 2/3 on gpsimd(Pool) in f32 -- keeps both far below PE's span.
  - out staging copies via nc.any (scheduler picks idle ACT/DVE).
"""

import math
import numpy as np

B, S, HID = 2, 2048, 2048
H, KVH, D = 16, 4, 128
GROUPS = 4            # head groups == KV heads
HD_PER_G = 4          # query heads per group
N_CORES = 8
P = 128
HIDC = HID // P       # 16 hid chunks
W = 512               # matmul chunk (psum bank limit: 512 f32)
WC = S // W           # 4 s-chunks
SB = S // P           # 16 s-blocks of 128
KG = 8                # k-groups per DMA piece (HIDC/KG = 2 pieces)

F16 = np.float16
_CACHE = {}


def build_nc(repeat=1, loop_n=None, internal_inputs=False):
    """loop_n: if set, wrap the body in a hardware For_i loop (for timing).
    internal_inputs: declare inputs as internal DRAM (garbage data, no host
    upload) -- timing-only variant."""
    import contextlib
    import concourse.bass as bass
    import concourse.tile as tile
    from concourse import bacc, mybir
    from concourse.bass_isa import ReduceOp

    f32 = mybir.dt.float32
    f16 = mybir.dt.float16

    nc = bacc.Bacc("TRN2", target_bir_lowering=False, debug=False,
                   num_devices=N_CORES)

    def din(name, shape, dt):
        if internal_inputs:
            return nc.dram_tensor(name, shape, dt).ap()
        return nc.dram_tensor(name, shape, dt, kind="ExternalInput").ap()
    hT = din("hT", [HID, S], f16)
    wqT = din("wqT", [HID, HD_PER_G * D], f16)
    wkvT = din("wkvT", [HID, 2 * D], f16)      # [wk.T | wv.T] packed
    woT = din("woT", [HD_PER_G * D, HID], f16)
    csT = din("csT", [D, 2 * S], f16)          # [cos | signed-sin] packed
    miT = din("miT", [P, 2 * P], f16)          # [mask | identity] packed
    if internal_inputs:
        out = nc.dram_tensor("out", [S, HID], f16).ap()
        sink = nc.dram_tensor("sink", [P, 4], f16, kind="ExternalOutput").ap()
    else:
        out = nc.dram_tensor("out", [S, HID], f16, kind="ExternalOutput").ap()
        sink = None
    wsink = nc.dram_tensor("wsink", [1, 4], f16).ap()   # warmup DCE guard

    inv_sqrt_d = 1.0 / math.sqrt(D)
    NPC = HIDC // KG      # dma pieces per hid dim (2)

    with tile.TileContext(nc) as tc:
        with (
            tc.tile_pool(name="consts", bufs=1) as consts,
            tc.tile_pool(name="persist", bufs=1) as persist,
            tc.tile_pool(name="hpool", bufs=3) as hpool,
            tc.tile_pool(name="rope", bufs=3) as rope,
            tc.tile_pool(name="vtc", bufs=2) as vtc,
            tc.tile_pool(name="expp", bufs=8) as expp,
            tc.tile_pool(name="dsump", bufs=2) as dsump,
            tc.tile_pool(name="denp", bufs=3) as denp,
            tc.tile_pool(name="outp", bufs=3) as outp,
            tc.tile_pool(name="psqk", bufs=2, space="PSUM") as psqk,
            tc.tile_pool(name="pspj", bufs=2, space="PSUM") as pspj,
            tc.tile_pool(name="avt", bufs=4, space="PSUM") as avtp,
        ):
            # ---- constant tiles (DMAs emitted in startup order below) ----
            wqT_sb = consts.tile([P, HIDC, HD_PER_G * D], f16)
            wkvT_sb = consts.tile([P, HIDC, 2 * D], f16)
            cs_sb = consts.tile([P, 2 * S], f16)
            mi_sb = consts.tile([P, 2 * P], f16)
            woT_sb = consts.tile([P, HD_PER_G, HID], f16)
            cosT_sb = cs_sb[:, 0:S]
            sinT_sb = cs_sb[:, S:2 * S]
            mask_sb = mi_sb[:, 0:P]
            ident_sb = mi_sb[:, P:2 * P]

            h_sb = [None] * WC

            def dma_h(c):
                # h0/h1 on the qAct HWDGE queue (parallel to consts on qSP;
                # dispatched before any ACT compute). Later chunks on qSP --
                # each dma_start costs the dispatching sequencer ~600ns, and
                # ACT must not stall behind them once exps are in flight.
                h_sb[c] = hpool.tile([P, HIDC, W], f16, tag="h", name=f"h{c}")
                hre = hT.rearrange("(k p) s -> p k s", p=P)
                eng = nc.scalar if c < 2 else nc.sync
                for g in range(NPC):
                    ks = slice(g * KG, (g + 1) * KG)
                    eng.dma_start(
                        out=h_sb[c][:, ks, :],
                        in_=hre[:, ks, c * W:(c + 1) * W])

            # const DMAs in first-use order (each dma_start costs its
            # dispatching sequencer ~1.25us, so keep the count low, but put
            # the data the first QK needs in front): chunk-0 rope table
            # slice, head-0 wq columns, wk|wv, mask -- then the rest.
            wqre = wqT.rearrange("(c p) d -> p c d", p=P)
            cs3 = cs_sb.rearrange("p (a s) -> p a s", a=2)
            csT3 = csT.rearrange("d (a s) -> d a s", a=2)
            nc.sync.dma_start(out=cs3[:, :, 0:W], in_=csT3[:, :, 0:W])
            nc.sync.dma_start(out=wqT_sb[:, :, 0:D], in_=wqre[:, :, 0:D])
            nc.sync.dma_start(out=wkvT_sb, in_=wkvT.rearrange("(c p) d -> p c d", p=P))
            nc.sync.dma_start(out=mi_sb, in_=miT)
            for hd in range(1, HD_PER_G):
                nc.sync.dma_start(out=wqT_sb[:, :, hd * D:(hd + 1) * D],
                                  in_=wqre[:, :, hd * D:(hd + 1) * D])
            nc.sync.dma_start(out=cs3[:, :, W:], in_=csT3[:, :, W:])
            nc.sync.dma_start(out=woT_sb, in_=woT.rearrange("(m p) h -> p m h", p=P))

            # ---- persistent intermediates ----------------------------
            qrT_sb = persist.tile([P, HD_PER_G, S], f16)   # rotated qT per head
            krT_sb = persist.tile([P, S], f16)             # rotated kT
            v_nat = persist.tile([P, SB, D], f16)          # v natural [sj, d]
            xT_sb = persist.tile([P, HD_PER_G, S], f16)    # attn out (transposed)
            warm_sb = persist.tile([P, 2 * P], f16)        # warmup operand
            wsink_sb = persist.tile([1, 4], f16)
            nc.vector.memset(warm_sb, 0.125)

            def pe_warmup(n):
                """Dummy matmuls to hold the PE HAM clock-gate at full rate
                through the DMA-bound startup; sunk to scratch DRAM so DCE
                keeps them."""
                wps = psqk.tile([P, 2 * P], f32, tag="qk", name="warmps")
                for _ in range(n):
                    nc.tensor.matmul(wps, warm_sb[:, 0:P], warm_sb,
                                     start=True, stop=True,
                                     skip_group_check=True)
                nc.vector.tensor_copy(wsink_sb, wps[0:1, 0:4])
                nc.sync.dma_start(out=wsink, in_=wsink_sb)

            if internal_inputs:
                # timing-only: fill internal inputs with finite values
                zb = consts.tile([P, S], f16, tag="zb")
                nc.vector.memset(zb, 0.01)
                zf1 = consts.tile([P, S], f16, tag="zf1")
                nc.vector.memset(zf1, 1.0)
                zf0 = consts.tile([P, S], f16, tag="zf0")
                nc.vector.memset(zf0, 0.0)
                for cc in range(HIDC):
                    hrc = hT.rearrange("(c p) s -> c p s", p=P)
                    nc.sync.dma_start(out=hrc[cc], in_=zb)
                    nc.sync.dma_start(
                        out=wqT.rearrange("(c p) d -> c p d", p=P)[cc],
                        in_=zb[:, 0:HD_PER_G * D])
                    nc.sync.dma_start(
                        out=wkvT.rearrange("(c p) d -> c p d", p=P)[cc],
                        in_=zb[:, 0:2 * D])
                for cc in range(HD_PER_G):
                    nc.sync.dma_start(
                        out=woT.rearrange("(m p) h -> m p h", p=P)[cc], in_=zb)
                nc.sync.dma_start(out=csT[:, 0:S], in_=zf1[:D, :])
                nc.sync.dma_start(out=csT[:, S:], in_=zf0[:D, :])
                nc.sync.dma_start(out=miT, in_=zb[:, 0:2 * P])

            def rope_chunk(ps, dst_ap, c):
                """dst = ps*cos + rot_half(ps)*sin_signed on wide chunk c.
                ps is f32 PSUM. One ACT copy to fp16, then all-fp16 DVE ops
                (2x mode); rotate-half via partition-shifted TT reads."""
                sl = slice(c * W, (c + 1) * W)
                q16 = rope.tile([P, W], f16, tag="q16")
                nc.scalar.copy(q16, ps)
                t2 = rope.tile([P, W], f16, tag="t2")
                nc.vector.tensor_copy(t2[0:64, :], q16[64:128, :])
                nc.vector.tensor_copy(t2[64:128, :], q16[0:64, :])
                t1 = rope.tile([P, W], f16, tag="t1")
                nc.vector.tensor_mul(t1, q16, cosT_sb[:, sl])
                nc.vector.tensor_mul(t2, t2, sinT_sb[:, sl])
                nc.vector.tensor_add(dst_ap, t1, t2)

            def proj_units(c):
                """List of emit-closures for chunk c's projections."""
                ssl = slice(c * W, (c + 1) * W)

                def mm_accum(w_sb, dsl, ps):
                    for k in range(HIDC):
                        nc.tensor.matmul(
                            ps, w_sb[:, k, dsl], h_sb[c][:, k, :],
                            start=(k == 0), stop=(k == HIDC - 1))

                def q_unit(hd):
                    def emit():
                        ps = pspj.tile([P, W], f32, tag="pj")
                        mm_accum(wqT_sb, slice(hd * D, (hd + 1) * D), ps)
                        rope_chunk(ps, qrT_sb[:, hd, ssl], c)
                    return emit

                def k_unit():
                    def emit():
                        ps = pspj.tile([P, W], f32, tag="pj")
                        mm_accum(wkvT_sb, slice(0, D), ps)
                        rope_chunk(ps, krT_sb[:, ssl], c)
                    return emit

                def v_unit():
                    def emit():
                        ps = pspj.tile([P, W], f32, tag="pj")
                        mm_accum(wkvT_sb, slice(D, 2 * D), ps)
                        vT_c = vtc.tile([P, W], f16, tag="vt")
                        nc.vector.tensor_copy(vT_c, ps)
                        for j in range(W // P):
                            jb = c * (W // P) + j
                            pst = pspj.tile([P, P], f16, tag="pj",
                                            name=f"pst{c}_{j}")
                            nc.tensor.transpose(
                                pst, vT_c[:, j * P:(j + 1) * P], ident_sb)
                            nc.vector.tensor_copy(v_nat[:, jb, :], pst)
                    return emit

                return [q_unit(0), k_unit(), v_unit(),
                        q_unit(1), q_unit(2), q_unit(3)]

            def attention_round(c, units):
                """Attention for si-chunk c, head-interleaved per key-block,
                with `units` (next chunk's proj closures) spread between
                key-block rounds."""
                base = c * W
                jbmax = (c + 1) * (W // P)
                avT = [avtp.tile([P, W], f32, tag="avt", name=f"avt{c}_{h}")
                       for h in range(HD_PER_G)]
                dsum = [dsump.tile([P, W], f16 if hd < 2 else f32,
                                   tag=f"ds{hd}", name=f"ds{c}_{hd}")
                        for hd in range(HD_PER_G)]
                nu = len(units)
                emitted = 0
                if nu:          # one unit up front: fill the round-boundary
                    units[0]()  # stall while avT banks cycle through the
                    emitted = 1  # previous round's normalize chain
                for jb in range(jbmax):
                    si_start = max(base, jb * P)
                    off = si_start - base
                    wd = W - off
                    diag = jb * P >= base
                    ets = []
                    for hd in range(HD_PER_G):
                        pss = psqk.tile([P, W], f32, tag="qk")
                        nc.tensor.matmul(
                            pss[:, :wd],
                            krT_sb[:, jb * P:(jb + 1) * P],
                            qrT_sb[:, hd, si_start:base + W],
                            start=True, stop=True)
                        et = expp.tile([P, W], f16, tag="expT")
                        nc.scalar.activation(
                            et[:, :wd], pss[:, :wd],
                            func=mybir.ActivationFunctionType.Exp,
                            scale=inv_sqrt_d)
                        if diag:  # diagonal block: causal mask
                            nc.vector.tensor_mul(et[:, 0:P], et[:, 0:P], mask_sb)
                        ets.append(et)
                    for hd in range(HD_PER_G):
                        et = ets[hd]
                        nc.tensor.matmul(
                            avT[hd][:, off:], v_nat[:, jb, :], et[:, :wd],
                            start=(jb == 0), stop=(jb == jbmax - 1),
                            skip_group_check=True)
                        eng = nc.vector if hd < 2 else nc.gpsimd
                        if jb == 0:
                            eng.tensor_copy(dsum[hd], et)
                        else:
                            eng.tensor_add(
                                dsum[hd][:, off:], dsum[hd][:, off:], et[:, :wd])
                    # spread next-chunk proj emission across jb rounds
                    want = max(emitted, (nu * (jb + 1)) // jbmax)
                    while emitted < want:
                        units[emitted]()
                        emitted += 1
                from concourse.bass_isa import ReduceOp as _R
                for hd in range(HD_PER_G):
                    den = denp.tile([P, W], f32, tag="den")
                    nc.gpsimd.partition_all_reduce(den, dsum[hd], P, _R.add)
                    nc.vector.reciprocal(den, den)
                    nc.vector.tensor_mul(
                        xT_sb[:, hd, base:base + W], avT[hd], den)

            def outproj_units(c, eager_dma=False):
                def sb_unit(sb):
                    def emit():
                        out_t = outp.tile([P, HID], f16, tag="out",
                                          name=f"out{sb}")
                        for j in range(HID // W):
                            pso = pspj.tile([P, W], f32, tag="pj")
                            for m in range(HD_PER_G):
                                nc.tensor.matmul(
                                    pso, xT_sb[:, m, sb * P:(sb + 1) * P],
                                    woT_sb[:, m, j * W:(j + 1) * W],
                                    start=(m == 0), stop=(m == HD_PER_G - 1))
                            nc.any.tensor_copy(out_t[:, j * W:(j + 1) * W], pso)
                            if eager_dma:
                                nc.sync.dma_start(
                                    out=out[sb * P:(sb + 1) * P,
                                            j * W:(j + 1) * W],
                                    in_=out_t[:, j * W:(j + 1) * W])
                        if not eager_dma:
                            nc.sync.dma_start(
                                out=out[sb * P:(sb + 1) * P, :], in_=out_t)
                    return emit
                return [sb_unit(sb)
                        for sb in range(c * (W // P), (c + 1) * (W // P))]

            def interleave(a, b):
                res, ia, ib = [], 0, 0
                while ia < len(a) or ib < len(b):
                    if ia < len(a):
                        res.append(a[ia]); ia += 1
                    if ib < len(b):
                        res.append(b[ib]); ib += 1
                return res

            loop_cm = (tc.For_i(0, loop_n, 1) if loop_n is not None
                       else contextlib.nullcontext())
            with loop_cm:
              for _rep in range(repeat):
                dma_h(0)
                dma_h(1)
                pe_warmup(24)
                for u in proj_units(0):
                    u()
                # fill-unit plan: rounds 1-2 are covered by their proj units
                # alone; round 3 (no proj left, ACT-bound) gets outproj(0..2).
                for c in range(WC):
                    if c < WC - 2:
                        dma_h(c + 2)
                    if c < WC - 1:
                        units = proj_units(c + 1)
                        if c == 2:
                            units = interleave(units, outproj_units(0))
                    else:
                        units = interleave(outproj_units(1), outproj_units(2))
                    attention_round(c, units)
                for u in outproj_units(WC - 1, eager_dma=True):
                    u()

            if sink is not None:
                nc.sync.dma_start(out=sink, in_=out[0:P, 0:4])

    nc.compile()
    return nc


def _prep_inputs(hidden_states, cos, sin, wq, wk, wv, wo):
    """Host-side shard + layout prep. Returns in_maps for cores 0..7."""
    hidden_states = np.asarray(hidden_states, dtype=np.float32)
    cos = np.asarray(cos, dtype=np.float32)
    sin = np.asarray(sin, dtype=np.float32)
    wq = np.asarray(wq, dtype=np.float32)
    wk = np.asarray(wk, dtype=np.float32)
    wv = np.asarray(wv, dtype=np.float32)
    wo = np.asarray(wo, dtype=np.float32)

    cosT = cos[:, 0, :].T                                       # [D, S]
    sinT_full = sin[:, 0, :].T                                  # [D, S]
    sinT = np.concatenate([-sinT_full[:64], sinT_full[64:]], axis=0)
    csT = np.ascontiguousarray(
        np.concatenate([cosT, sinT], axis=1)).astype(F16)       # [D, 2S]

    mask = (np.arange(P)[:, None] <= np.arange(P)[None, :]).astype(np.float32)
    miT = np.ascontiguousarray(
        np.concatenate([mask, np.eye(P)], axis=1)).astype(F16)  # [P, 2P]

    hTs = [np.ascontiguousarray(hidden_states[b].T).astype(F16)
           for b in range(B)]

    in_maps = []
    for core in range(N_CORES):
        b, g = divmod(core, GROUPS)
        qsl = slice(g * HD_PER_G * D, (g + 1) * HD_PER_G * D)
        ksl = slice(g * D, (g + 1) * D)
        wkv = np.concatenate([wk[ksl, :].T, wv[ksl, :].T], axis=1)
        in_maps.append({
            "hT": hTs[b],
            "wqT": np.ascontiguousarray(wq[qsl, :].T).astype(F16),
            "wkvT": np.ascontiguousarray(wkv).astype(F16),
            "woT": np.ascontiguousarray(wo[:, qsl].T).astype(F16),
            "csT": csT,
            "miT": miT,
        })
    return in_maps


def kernel(hidden_states, cos, sin, wq, wk, wv, wo):
    from concourse.bass_utils import run_bass_kernel_spmd

    if "nc" not in _CACHE:
        _CACHE["nc"] = build_nc()
    nc = _CACHE["nc"]

    in_maps = _prep_inputs(hidden_states, cos, sin, wq, wk, wv, wo)
    res = run_bass_kernel_spmd(nc, in_maps, core_ids=list(range(N_CORES)))

    out = np.zeros((B, S, HID), dtype=np.float32)
    for core in range(N_CORES):
        b = core // GROUPS
        out[b] += res.results[core]["out"].astype(np.float32)
    return out


# revision 7
# speedup vs baseline: 1.0783x; 1.0783x over previous
"""GQA causal attention (B=2,S=2048,HID=2048,H=16,KVH=4,D=128) on 8 trn2 cores.

Sharding: core = b*4 + g  (b: batch, g: head-group of 4 Q heads + 1 KV head).
Per-core kernel computes q/k/v projections (+RoPE), causal softmax attention
for its 4 heads, and a partial output projection; host sums the 4 partials
per batch.

v2 (pipelined, fp16):
  - All matmul operands fp16 (same PE speed as bf16, ~8x mantissa) with fp32
    PSUM accumulation; output partials fp16, host sums in f32.
  - Startup DMAs split into 4-k-group pieces so the first projection matmuls
    start ~2us in instead of waiting for whole-tensor loads.
  - Attention emitted head-interleaved per key-block (4 independent
    QK->exp->AV chains; avT holds 4 PSUM banks) so PE never stalls on the
    ACT exp of a single chain.
  - proj(c+1) emission spread between attention(c) key-block rounds: PE has
    dense independent work while ACT drains the exp queue.
  - softmax denominator accumulation: heads 0/1 on DVE in fp16 (2x mode),
    heads 2/3 on gpsimd(Pool) in f32 -- keeps both far below PE's span.
  - out staging copies via nc.any (scheduler picks idle ACT/DVE).
"""

import math
import numpy as np

B, S, HID = 2, 2048, 2048
H, KVH, D = 16, 4, 128
GROUPS = 4            # head groups == KV heads
HD_PER_G = 4          # query heads per group
N_CORES = 8
P = 128
HIDC = HID // P       # 16 hid chunks
W = 512               # matmul chunk (psum bank limit: 512 f32)
WC = S // W           # 4 s-chunks
SB = S // P           # 16 s-blocks of 128
KG = 8                # k-groups per DMA piece (HIDC/KG = 2 pieces)

F16 = np.float16
_CACHE = {}


def build_nc(repeat=1, loop_n=None, internal_inputs=False):
    """loop_n: if set, wrap the body in a hardware For_i loop (for timing).
    internal_inputs: declare inputs as internal DRAM (garbage data, no host
    upload) -- timing-only variant."""
    import contextlib
    import concourse.bass as bass
    import concourse.tile as tile
    from concourse import bacc, mybir
    from concourse.bass_isa import ReduceOp

    f32 = mybir.dt.float32
    f16 = mybir.dt.float16

    nc = bacc.Bacc("TRN2", target_bir_lowering=False, debug=False,
                   num_devices=N_CORES)

    def din(name, shape, dt):
        if internal_inputs:
            return nc.dram_tensor(name, shape, dt).ap()
        return nc.dram_tensor(name, shape, dt, kind="ExternalInput").ap()
    hT = din("hT", [HID, S], f16)
    wqT = din("wqT", [HID, HD_PER_G * D], f16)
    wkvT = din("wkvT", [HID, 2 * D], f16)      # [wk.T | wv.T] packed
    woT = din("woT", [HD_PER_G * D, HID], f16)
    csT = din("csT", [D, 2 * S], f16)          # [cos | signed-sin] packed
    miT = din("miT", [P, 2 * P], f16)          # [mask | identity] packed
    if internal_inputs:
        out = nc.dram_tensor("out", [S, HID], f16).ap()
        sink = nc.dram_tensor("sink", [P, 4], f16, kind="ExternalOutput").ap()
    else:
        out = nc.dram_tensor("out", [S, HID], f16, kind="ExternalOutput").ap()
        sink = None
    wsink = nc.dram_tensor("wsink", [1, 4], f16).ap()   # warmup DCE guard

    inv_sqrt_d = 1.0 / math.sqrt(D)
    NPC = HIDC // KG      # dma pieces per hid dim (2)

    with tile.TileContext(nc) as tc:
        with (
            tc.tile_pool(name="consts", bufs=1) as consts,
            tc.tile_pool(name="persist", bufs=1) as persist,
            tc.tile_pool(name="hpool", bufs=3) as hpool,
            tc.tile_pool(name="rope", bufs=3) as rope,
            tc.tile_pool(name="vtc", bufs=2) as vtc,
            tc.tile_pool(name="expp", bufs=12) as expp,
            tc.tile_pool(name="dsump", bufs=2) as dsump,
            tc.tile_pool(name="denp", bufs=3) as denp,
            tc.tile_pool(name="outp", bufs=3) as outp,
            tc.tile_pool(name="psqk", bufs=2, space="PSUM") as psqk,
            tc.tile_pool(name="pspj", bufs=2, space="PSUM") as pspj,
            tc.tile_pool(name="avt", bufs=4, space="PSUM") as avtp,
        ):
            # ---- constant tiles (DMAs emitted in startup order below) ----
            wqT_sb = consts.tile([P, HIDC, HD_PER_G * D], f16)
            wkvT_sb = consts.tile([P, HIDC, 2 * D], f16)
            cs_sb = consts.tile([P, 2 * S], f16)
            mi_sb = consts.tile([P, 2 * P], f16)
            woT_sb = consts.tile([P, HD_PER_G, HID], f16)
            cosT_sb = cs_sb[:, 0:S]
            sinT_sb = cs_sb[:, S:2 * S]
            mask_sb = mi_sb[:, 0:P]
            ident_sb = mi_sb[:, P:2 * P]

            h_sb = [None] * WC

            def dma_h(c):
                # h0/h1 on the qAct HWDGE queue (parallel to consts on qSP;
                # dispatched before any ACT compute). Later chunks on qSP --
                # each dma_start costs the dispatching sequencer ~600ns, and
                # ACT must not stall behind them once exps are in flight.
                h_sb[c] = hpool.tile([P, HIDC, W], f16, tag="h", name=f"h{c}")
                hre = hT.rearrange("(k p) s -> p k s", p=P)
                eng = nc.scalar if c < 2 else nc.sync
                for g in range(NPC):
                    ks = slice(g * KG, (g + 1) * KG)
                    eng.dma_start(
                        out=h_sb[c][:, ks, :],
                        in_=hre[:, ks, c * W:(c + 1) * W])

            # const DMAs in first-use order (each dma_start costs its
            # dispatching sequencer ~1.25us, so keep the count low, but put
            # the data the first QK needs in front): chunk-0 rope table
            # slice, head-0 wq columns, wk|wv, mask -- then the rest.
            wqre = wqT.rearrange("(c p) d -> p c d", p=P)
            cs3 = cs_sb.rearrange("p (a s) -> p a s", a=2)
            csT3 = csT.rearrange("d (a s) -> d a s", a=2)
            nc.sync.dma_start(out=cs3[:, :, 0:W], in_=csT3[:, :, 0:W])
            nc.sync.dma_start(out=wqT_sb[:, :, 0:D], in_=wqre[:, :, 0:D])
            nc.sync.dma_start(out=wkvT_sb, in_=wkvT.rearrange("(c p) d -> p c d", p=P))
            nc.sync.dma_start(out=mi_sb, in_=miT)
            for hd in range(1, HD_PER_G):
                nc.sync.dma_start(out=wqT_sb[:, :, hd * D:(hd + 1) * D],
                                  in_=wqre[:, :, hd * D:(hd + 1) * D])
            nc.sync.dma_start(out=cs3[:, :, W:], in_=csT3[:, :, W:])
            nc.sync.dma_start(out=woT_sb, in_=woT.rearrange("(m p) h -> p m h", p=P))

            # ---- persistent intermediates ----------------------------
            qrT_sb = persist.tile([P, HD_PER_G, S], f16)   # rotated qT per head
            krT_sb = persist.tile([P, S], f16)             # rotated kT
            v_nat = persist.tile([P, SB, D], f16)          # v natural [sj, d]
            xT_sb = persist.tile([P, HD_PER_G, S], f16)    # attn out (transposed)
            warm_sb = persist.tile([P, 2 * P], f16)        # warmup operand
            wsink_sb = persist.tile([1, 4], f16)
            nc.vector.memset(warm_sb, 0.125)

            def pe_warmup(n):
                """Dummy matmuls to hold the PE HAM clock-gate at full rate
                through the DMA-bound startup; sunk to scratch DRAM so DCE
                keeps them."""
                wps = psqk.tile([P, 2 * P], f32, tag="qk", name="warmps")
                for _ in range(n):
                    nc.tensor.matmul(wps, warm_sb[:, 0:P], warm_sb,
                                     start=True, stop=True,
                                     skip_group_check=True)
                nc.vector.tensor_copy(wsink_sb, wps[0:1, 0:4])
                nc.sync.dma_start(out=wsink, in_=wsink_sb)

            if internal_inputs:
                # timing-only: fill internal inputs with finite values
                zb = consts.tile([P, S], f16, tag="zb")
                nc.vector.memset(zb, 0.01)
                zf1 = consts.tile([P, S], f16, tag="zf1")
                nc.vector.memset(zf1, 1.0)
                zf0 = consts.tile([P, S], f16, tag="zf0")
                nc.vector.memset(zf0, 0.0)
                for cc in range(HIDC):
                    hrc = hT.rearrange("(c p) s -> c p s", p=P)
                    nc.sync.dma_start(out=hrc[cc], in_=zb)
                    nc.sync.dma_start(
                        out=wqT.rearrange("(c p) d -> c p d", p=P)[cc],
                        in_=zb[:, 0:HD_PER_G * D])
                    nc.sync.dma_start(
                        out=wkvT.rearrange("(c p) d -> c p d", p=P)[cc],
                        in_=zb[:, 0:2 * D])
                for cc in range(HD_PER_G):
                    nc.sync.dma_start(
                        out=woT.rearrange("(m p) h -> m p h", p=P)[cc], in_=zb)
                nc.sync.dma_start(out=csT[:, 0:S], in_=zf1[:D, :])
                nc.sync.dma_start(out=csT[:, S:], in_=zf0[:D, :])
                nc.sync.dma_start(out=miT, in_=zb[:, 0:2 * P])

            def rope_chunk(ps, dst_ap, c):
                """dst = ps*cos + rot_half(ps)*sin_signed on wide chunk c.
                ps is f32 PSUM. One ACT copy to fp16, then all-fp16 DVE ops
                (2x mode); rotate-half via partition-shifted TT reads."""
                sl = slice(c * W, (c + 1) * W)
                q16 = rope.tile([P, W], f16, tag="q16")
                nc.scalar.copy(q16, ps)
                t2 = rope.tile([P, W], f16, tag="t2")
                nc.vector.tensor_copy(t2[0:64, :], q16[64:128, :])
                nc.vector.tensor_copy(t2[64:128, :], q16[0:64, :])
                t1 = rope.tile([P, W], f16, tag="t1")
                nc.vector.tensor_mul(t1, q16, cosT_sb[:, sl])
                nc.vector.tensor_mul(t2, t2, sinT_sb[:, sl])
                nc.vector.tensor_add(dst_ap, t1, t2)

            def proj_units(c):
                """List of emit-closures for chunk c's projections."""
                ssl = slice(c * W, (c + 1) * W)

                def mm_accum(w_sb, dsl, ps):
                    for k in range(HIDC):
                        nc.tensor.matmul(
                            ps, w_sb[:, k, dsl], h_sb[c][:, k, :],
                            start=(k == 0), stop=(k == HIDC - 1))

                def q_unit(hd):
                    def emit():
                        ps = pspj.tile([P, W], f32, tag="pj")
                        mm_accum(wqT_sb, slice(hd * D, (hd + 1) * D), ps)
                        rope_chunk(ps, qrT_sb[:, hd, ssl], c)
                    return emit

                def k_unit():
                    def emit():
                        ps = pspj.tile([P, W], f32, tag="pj")
                        mm_accum(wkvT_sb, slice(0, D), ps)
                        rope_chunk(ps, krT_sb[:, ssl], c)
                    return emit

                def v_unit():
                    def emit():
                        ps = pspj.tile([P, W], f32, tag="pj")
                        mm_accum(wkvT_sb, slice(D, 2 * D), ps)
                        vT_c = vtc.tile([P, W], f16, tag="vt")
                        nc.vector.tensor_copy(vT_c, ps)
                        for j in range(W // P):
                            jb = c * (W // P) + j
                            pst = pspj.tile([P, P], f16, tag="pj",
                                            name=f"pst{c}_{j}")
                            nc.tensor.transpose(
                                pst, vT_c[:, j * P:(j + 1) * P], ident_sb)
                            nc.vector.tensor_copy(v_nat[:, jb, :], pst)
                    return emit

                return [q_unit(0), k_unit(), v_unit(),
                        q_unit(1), q_unit(2), q_unit(3)]

            def attention_round(c, units):
                """Attention for si-chunk c, head-interleaved per key-block,
                with `units` (next chunk's proj closures) spread between
                key-block rounds."""
                base = c * W
                jbmax = (c + 1) * (W // P)
                avT = [avtp.tile([P, W], f32, tag="avt", name=f"avt{c}_{h}")
                       for h in range(HD_PER_G)]
                dsum = [dsump.tile([P, W], f16 if hd < 2 else f32,
                                   tag=f"ds{hd}", name=f"ds{c}_{hd}")
                        for hd in range(HD_PER_G)]
                nu = len(units)
                emitted = 0
                if nu:          # one unit up front: fill the round-boundary
                    units[0]()  # stall while avT banks cycle through the
                    emitted = 1  # previous round's normalize chain
                for jb in range(jbmax):
                    si_start = max(base, jb * P)
                    off = si_start - base
                    wd = W - off
                    diag = jb * P >= base
                    ets = []
                    for hd in range(HD_PER_G):
                        pss = psqk.tile([P, W], f32, tag="qk")
                        nc.tensor.matmul(
                            pss[:, :wd],
                            krT_sb[:, jb * P:(jb + 1) * P],
                            qrT_sb[:, hd, si_start:base + W],
                            start=True, stop=True)
                        et = expp.tile([P, W], f16, tag="expT")
                        nc.scalar.activation(
                            et[:, :wd], pss[:, :wd],
                            func=mybir.ActivationFunctionType.Exp,
                            scale=inv_sqrt_d)
                        if diag:  # diagonal block: causal mask
                            nc.vector.tensor_mul(et[:, 0:P], et[:, 0:P], mask_sb)
                        ets.append(et)
                    for hd in range(HD_PER_G):
                        et = ets[hd]
                        nc.tensor.matmul(
                            avT[hd][:, off:], v_nat[:, jb, :], et[:, :wd],
                            start=(jb == 0), stop=(jb == jbmax - 1),
                            skip_group_check=True)
                        eng = nc.vector if hd < 2 else nc.gpsimd
                        if jb == 0:
                            eng.tensor_copy(dsum[hd], et)
                        else:
                            eng.tensor_add(
                                dsum[hd][:, off:], dsum[hd][:, off:], et[:, :wd])
                    # spread next-chunk proj emission across jb rounds
                    want = max(emitted, (nu * (jb + 1)) // jbmax)
                    while emitted < want:
                        units[emitted]()
                        emitted += 1
                from concourse.bass_isa import ReduceOp as _R
                for hd in range(HD_PER_G):
                    den = denp.tile([P, W], f32, tag="den")
                    nc.gpsimd.partition_all_reduce(den, dsum[hd], P, _R.add)
                    nc.vector.reciprocal(den, den)
                    nc.vector.tensor_mul(
                        xT_sb[:, hd, base:base + W], avT[hd], den)

            def outproj_units(c, eager_dma=False):
                def sb_unit(sb):
                    def emit():
                        out_t = outp.tile([P, HID], f16, tag="out",
                                          name=f"out{sb}")
                        for j in range(HID // W):
                            pso = pspj.tile([P, W], f32, tag="pj")
                            for m in range(HD_PER_G):
                                nc.tensor.matmul(
                                    pso, xT_sb[:, m, sb * P:(sb + 1) * P],
                                    woT_sb[:, m, j * W:(j + 1) * W],
                                    start=(m == 0), stop=(m == HD_PER_G - 1))
                            nc.any.tensor_copy(out_t[:, j * W:(j + 1) * W], pso)
                            if eager_dma:
                                nc.sync.dma_start(
                                    out=out[sb * P:(sb + 1) * P,
                                            j * W:(j + 1) * W],
                                    in_=out_t[:, j * W:(j + 1) * W])
                        if not eager_dma:
                            nc.sync.dma_start(
                                out=out[sb * P:(sb + 1) * P, :], in_=out_t)
                    return emit
                return [sb_unit(sb)
                        for sb in range(c * (W // P), (c + 1) * (W // P))]

            def interleave(a, b):
                res, ia, ib = [], 0, 0
                while ia < len(a) or ib < len(b):
                    if ia < len(a):
                        res.append(a[ia]); ia += 1
                    if ib < len(b):
                        res.append(b[ib]); ib += 1
                return res

            loop_cm = (tc.For_i(0, loop_n, 1) if loop_n is not None
                       else contextlib.nullcontext())
            with loop_cm:
              for _rep in range(repeat):
                dma_h(0)
                dma_h(1)
                pe_warmup(24)
                for iu, u in enumerate(proj_units(0)):
                    u()
                    if iu < 2:  # keep PE warm through the DMA-bound start
                        pe_warmup(12)
                # fill-unit plan: rounds 1-2 are covered by their proj units
                # alone; round 3 (no proj left, ACT-bound) gets outproj(0..2).
                for c in range(WC):
                    if c < WC - 2:
                        dma_h(c + 2)
                    if c < WC - 1:
                        units = proj_units(c + 1)
                        if c == 2:
                            units = interleave(units, outproj_units(0))
                    else:
                        units = interleave(outproj_units(1), outproj_units(2))
                    attention_round(c, units)
                for u in outproj_units(WC - 1):
                    u()

            if sink is not None:
                nc.sync.dma_start(out=sink, in_=out[0:P, 0:4])

    nc.compile()
    return nc


def _prep_inputs(hidden_states, cos, sin, wq, wk, wv, wo):
    """Host-side shard + layout prep. Returns in_maps for cores 0..7."""
    hidden_states = np.asarray(hidden_states, dtype=np.float32)
    cos = np.asarray(cos, dtype=np.float32)
    sin = np.asarray(sin, dtype=np.float32)
    wq = np.asarray(wq, dtype=np.float32)
    wk = np.asarray(wk, dtype=np.float32)
    wv = np.asarray(wv, dtype=np.float32)
    wo = np.asarray(wo, dtype=np.float32)

    cosT = cos[:, 0, :].T                                       # [D, S]
    sinT_full = sin[:, 0, :].T                                  # [D, S]
    sinT = np.concatenate([-sinT_full[:64], sinT_full[64:]], axis=0)
    csT = np.ascontiguousarray(
        np.concatenate([cosT, sinT], axis=1)).astype(F16)       # [D, 2S]

    mask = (np.arange(P)[:, None] <= np.arange(P)[None, :]).astype(np.float32)
    miT = np.ascontiguousarray(
        np.concatenate([mask, np.eye(P)], axis=1)).astype(F16)  # [P, 2P]

    hTs = [np.ascontiguousarray(hidden_states[b].T).astype(F16)
           for b in range(B)]

    in_maps = []
    for core in range(N_CORES):
        b, g = divmod(core, GROUPS)
        qsl = slice(g * HD_PER_G * D, (g + 1) * HD_PER_G * D)
        ksl = slice(g * D, (g + 1) * D)
        wkv = np.concatenate([wk[ksl, :].T, wv[ksl, :].T], axis=1)
        in_maps.append({
            "hT": hTs[b],
            "wqT": np.ascontiguousarray(wq[qsl, :].T).astype(F16),
            "wkvT": np.ascontiguousarray(wkv).astype(F16),
            "woT": np.ascontiguousarray(wo[:, qsl].T).astype(F16),
            "csT": csT,
            "miT": miT,
        })
    return in_maps


def kernel(hidden_states, cos, sin, wq, wk, wv, wo):
    from concourse.bass_utils import run_bass_kernel_spmd

    if "nc" not in _CACHE:
        _CACHE["nc"] = build_nc()
    nc = _CACHE["nc"]

    in_maps = _prep_inputs(hidden_states, cos, sin, wq, wk, wv, wo)
    res = run_bass_kernel_spmd(nc, in_maps, core_ids=list(range(N_CORES)))

    out = np.zeros((B, S, HID), dtype=np.float32)
    for core in range(N_CORES):
        b = core // GROUPS
        out[b] += res.results[core]["out"].astype(np.float32)
    return out
